# revision 21
# baseline (speedup 1.0000x reference)
"""Deformable conv (AdaptiveConv) Trainium2 Bass kernel, 8-core data-parallel.

Strategy per core (each core owns half an image = 2048 output pixels):
  - x is host-relaid to a row-pair-interleaved, x-padded pixel-major bf16
    image: record s = (66*y + x + 1)*2 + r holds channels of pixel (y+r, x).
    One 2KB dma_gather descriptor at v = (66*y0 + x0 + 1)*2 then fetches all
    FOUR bilinear corners (y0/y1 x x0/x1) of one (pixel, tap) sample — this
    halves the GPSIMD descriptor-generation work, which profiling showed to
    be the bottleneck (~7ns of Q7 time per gathered index).
  - Sample positions / bilinear weights are computed on-device (DVE) from the
    offset tensor; corner indices become int16 dma_gather indices via a
    16-partition fold + replication.
  - The 4-corner blend runs on TensorE: matmuls against per-128-px-group
    diagonal weight matrices D_j = I * wv_j (one tensor_scalar each, built on
    DVE/ACT), accumulating sampled features S[c, px] in PSUM.
  - The 3x3x256 conv is 18 accumulated matmuls per 256-px block with host
    pre-transposed bf16 weights; ReLU on ScalarE; f32 out.
"""
import numpy as np
import ml_dtypes

import concourse.bass as bass
import concourse.mybir as mybir
from concourse.tile import TileContext
from concourse import bass_utils
import concourse.bacc as bacc

F32 = mybir.dt.float32
BF16 = mybir.dt.bfloat16
I16 = mybir.dt.int16
I32 = mybir.dt.int32
OP = mybir.AluOpType
ACTF = mybir.ActivationFunctionType

# problem constants
N, C, H, W, CO, K2 = 4, 256, 64, 64, 256, 9
NCORES = 8
PXC = 2048          # output pixels per core (32 rows)
ROWSC = 32          # rows per core
NCALLS = 8          # 256-px blocks per core
XREC = 8704         # records in the interleaved padded image (rows y=-1..63)
VMAX2 = 8578        # max gatherable record index (elem covers v..v+3)
GROWS = VMAX2 + 1   # gather-source row count

_CACHE = {}
DBG_CALLS = NCALLS
DBG_SP = True


def _build_program():
    nc = bacc.Bacc('TRN2', num_devices=NCORES, num_swdge_queues=4)

    d_xq = nc.dram_tensor('xq', [XREC * C], BF16, kind='ExternalInput')
    d_wt = nc.dram_tensor('wt', [128, K2 * 2 * 2 * 128], BF16, kind='ExternalInput')
    d_inB = nc.dram_tensor('inB', [128, 4 * 192], F32, kind='ExternalInput')
    d_inA = nc.dram_tensor('inA', [128, 4 * 144], F32, kind='ExternalInput')
    d_mask = nc.dram_tensor('maskBK', [128, 128 * 18], BF16, kind='ExternalInput')
    d_out = nc.dram_tensor('out', [CO, PXC], F32, kind='ExternalOutput')

    gather_src = bass.AP(d_xq, 0, [[C, GROWS], [1, 4 * C]])

    with TileContext(nc) as tc:
        with tc.tile_pool(name='const', bufs=1) as cpool, \
             tc.tile_pool(name='pipe', bufs=1) as ppool, \
             tc.tile_pool(name='gp', bufs=8) as gpool, \
             tc.tile_pool(name='sp', bufs=3) as spool, \
             tc.tile_pool(name='dp', bufs=2) as dpool, \
             tc.tile_pool(name='op', bufs=2) as opool, \
             tc.tile_pool(name='pb', bufs=4, space='PSUM') as pbpool, \
             tc.tile_pool(name='po', bufs=2, space='PSUM') as popool:

            def load(dram, shape, dtype, pool=cpool):
                t = pool.tile(shape, dtype, tag=dram.name + '_t')
                nc.sync.dma_start(t[:], dram.ap())
                return t

            from concourse import library_config
            nc.gpsimd.load_library(library_config.mlp)

            # tiny warmup gathers: touch all 4 SWDGE queues early so the
            # first real gathers don't pay Q7 icache/queue cold-start.
            # idx tile comes from an on-device iota (no DMA dependency) and
            # each queue gets its own output tile (no WAW serialization).
            t_widx = cpool.tile([128, 1], I16, tag='widx_t')
            nc.gpsimd.iota(t_widx[:], [[0, 1]], base=0, channel_multiplier=0)
            warm_src = bass.AP(d_xq, 0, [[C, 16], [1, 128]])
            for q in range(4):
                t_warm = cpool.tile([128, 1, 128], BF16, tag=f'warm{q}')
                nc.gpsimd.dma_gather(
                    t_warm[:], warm_src, t_widx[:], 16, 16, 128,
                    elem_step=C, queue_num=q)

            t_inB = load(d_inB, [128, 4, 192], F32)
            t_offBy, t_offBx = t_inB[:, 0, :], t_inB[:, 1, :]
            t_bBy, t_bBx = t_inB[:, 2, :], t_inB[:, 3, :]
            t_inA = load(d_inA, [128, 4, 144], F32)
            t_offAy, t_offAx = t_inA[:, 0, :], t_inA[:, 1, :]
            t_bAy, t_bAx = t_inA[:, 2, :], t_inA[:, 3, :]
            t_wt = load(d_wt, [128, K2 * 2 * 2 * 128], BF16)
            t_mask = load(d_mask, [128, 18, 128], BF16)

            def floorp(z, cols, tag):
                """floor of shifted-positive z (exact, cast-mode independent)."""
                ti = ppool.tile([128, cols], I32, tag=f'{tag}_i')
                nc.vector.tensor_copy(ti[:], z[:])
                tf = ppool.tile([128, cols], F32, tag=f'{tag}_f')
                nc.vector.tensor_copy(tf[:], ti[:])
                ov = ppool.tile([128, cols], F32, tag=f'{tag}_ov')
                nc.vector.tensor_tensor(ov[:], tf[:], z[:], OP.is_gt)
                nc.vector.tensor_tensor(tf[:], tf[:], ov[:], OP.subtract)
                return tf

            # ---- layout-B pipeline: gather indices [128,192] ----
            # P = u*16 + q, u = kl*2 + jh (<6); C = cb*24 + kt*8 + pl
            pyB = ppool.tile([128, 192], F32, tag='pyB')
            nc.vector.tensor_tensor(pyB[:], t_offBy[:], t_bBy[:], OP.add)
            y0B = floorp(pyB, 192, 'y0B')
            pxB = ppool.tile([128, 192], F32, tag='pxB')
            nc.vector.tensor_tensor(pxB[:], t_offBx[:], t_bBx[:], OP.add)
            x0B = floorp(pxB, 192, 'x0B')
            # v = (66*(y0+1) + x0 + 1)*2 = 132*y0P + 2*x0P - 2010
            v = ppool.tile([128, 192], F32, tag='v')
            nc.vector.tensor_scalar(v[:], y0B[:], 132.0, None, OP.mult)
            v2 = ppool.tile([128, 192], F32, tag='v2')
            nc.vector.tensor_scalar(v2[:], x0B[:], 2.0, -2010.0, OP.mult, OP.add)
            nc.vector.tensor_tensor(v[:], v[:], v2[:], OP.add)
            nc.vector.tensor_scalar(v[:], v[:], 0.0, float(VMAX2), OP.max, OP.min)
            xb = ppool.tile([128, 192], I16, tag='xb16')
            nc.vector.tensor_copy(xb[:], v[:])

            # fold to Y [128, 8cb * 3kt * 48] int16; call (cb,kt): cols 48
            # within call: s = kl*16 + jh*8 + pl
            t_Y = cpool.tile([128, NCALLS * 3 * 48], I16, tag='Y')
            xbv = xb[:].rearrange('p (cb kt pl) -> p cb kt pl', cb=8, kt=3)
            yv4 = t_Y[:].rearrange('p (cb kt pl) -> p cb kt pl', cb=8, kt=3, pl=48)
            # build Y in two cb-halves so the first gathers start earlier
            for h in range(2):
                cbs = slice(h * 4, h * 4 + 4)
                for u in range(6):
                    kl, jh = u // 2, u % 2
                    base = kl * 16 + jh * 8
                    nc.sync.dma_start(
                        yv4[0:16, cbs, :, base:base + 8],
                        xbv[u * 16:u * 16 + 16, cbs])
                half = slice(h * 4 * 144, (h + 1) * 4 * 144)
                nc.sync.dma_start(t_Y[16:32, half], t_Y[0:16, half])
                nc.sync.dma_start(t_Y[32:64, half], t_Y[0:32, half])
                nc.sync.dma_start(t_Y[64:128, half], t_Y[0:64, half])

            # ---- layout-A pipeline: corner weights wv0..wv3 [128,144] ----
            # partition p = px % 128; col = jg*9 + k (jg = 128-px group)
            def axis_weights(toff, tbase, lo0, hi0, lo1, hi1, tag):
                pP = ppool.tile([128, 144], F32, tag=f'p{tag}')
                nc.vector.tensor_tensor(pP[:], toff[:], tbase[:], OP.add)
                f0 = floorp(pP, 144, f'f{tag}')
                fr = ppool.tile([128, 144], F32, tag=f'fr{tag}')
                nc.vector.tensor_tensor(fr[:], pP[:], f0[:], OP.subtract)
                w0 = ppool.tile([128, 144], F32, tag=f'w0{tag}')
                nc.vector.tensor_scalar(w0[:], fr[:], -1.0, 1.0, OP.mult, OP.add)
                m1 = ppool.tile([128, 144], F32, tag=f'm1{tag}')
                m2 = ppool.tile([128, 144], F32, tag=f'm2{tag}')
                nc.vector.tensor_scalar(m1[:], f0[:], lo0, None, OP.is_ge)
                nc.vector.tensor_scalar(m2[:], f0[:], hi0, None, OP.is_le)
                nc.vector.tensor_tensor(m1[:], m1[:], m2[:], OP.mult)
                nc.vector.tensor_tensor(w0[:], w0[:], m1[:], OP.mult)
                w1 = ppool.tile([128, 144], F32, tag=f'w1{tag}')
                nc.vector.tensor_scalar(m1[:], f0[:], lo1, None, OP.is_ge)
                nc.vector.tensor_scalar(m2[:], f0[:], hi1, None, OP.is_le)
                nc.vector.tensor_tensor(m1[:], m1[:], m2[:], OP.mult)
                nc.vector.tensor_tensor(w1[:], fr[:], m1[:], OP.mult)
                return w0, w1

            wy0, wy1 = axis_weights(t_offAy, t_bAy, 16.0, 79.0, 15.0, 78.0, 'ya')
            wx0, wx1 = axis_weights(t_offAx, t_bAx, 16.0, 79.0, 15.0, 78.0, 'xa')
            # corner order in the gathered elem: (y0x0),(y1x0),(y0x1),(y1x1)
            wv = []
            for j, (wy, wx) in enumerate(
                    [(wy0, wx0), (wy1, wx0), (wy0, wx1), (wy1, wx1)]):
                t = cpool.tile([128, 144], BF16, tag=f'wv{j}')
                nc.vector.tensor_tensor(t[:], wy[:], wx[:], OP.mult)
                wv.append(t)

            # ---- main loop over 256-px blocks ----
            for cb in range(DBG_CALLS):
                t_Gs = []
                for kt in range(3):
                    # 768 idxs: taps 3kt..3kt+2; tile block j = kl*2 + jh
                    t_Gk = gpool.tile([128, 6, 1024], BF16, tag='G')
                    nc.gpsimd.dma_gather(
                        t_Gk[:], gather_src,
                        t_Y[:, (cb * 3 + kt) * 48:(cb * 3 + kt + 1) * 48],
                        768, 768, 4 * C, elem_step=C, single_packet=DBG_SP,
                        queue_num=(cb * 3 + kt) % 4)
                    t_Gs.append(t_Gk)

                # batched diag build: D_j[p, kk, a'] = (p==a') * wv_j[p, (cb*2+jh)*9+k]
                Ds = []
                for j in range(4):
                    D = dpool.tile([128, 18, 128], BF16, tag=f'D{j}')
                    nc.vector.tensor_tensor(
                        D[:], t_mask[:],
                        wv[j][:, cb * 18:(cb + 1) * 18].unsqueeze(2)
                        .to_broadcast([128, 18, 128]),
                        OP.mult)
                    Ds.append(D)

                t_S = spool.tile([128, K2, 2, 256], BF16, tag='S')
                for k in range(K2):
                    pb = pbpool.tile([128, 512], F32, tag='pb')
                    for jh in range(2):
                        kk = jh * K2 + k
                        for ct in range(2):
                            sl = slice(ct * 256 + jh * 128, ct * 256 + jh * 128 + 128)
                            for j in range(4):
                                nc.tensor.matmul(
                                    pb[:, sl],
                                    t_Gs[k // 3][:, (k % 3) * 2 + jh,
                                                 j * 256 + ct * 128:
                                                 j * 256 + ct * 128 + 128],
                                    Ds[j][:, kk, :].squeeze(),
                                    start=(j == 0), stop=(j == 3))
                    pbv = pb[:].rearrange('p (a b) -> p a b', a=2)
                    nc.scalar.activation(t_S[:, k, :, :], pbv, ACTF.Copy)

                po = popool.tile([128, 512], F32, tag='po')
                for ot in range(2):
                    for ki in range(18):
                        k, ct = divmod(ki, 2)
                        wcol = (k * 2 + ct) * 2 + ot
                        nc.tensor.matmul(
                            po[:, ot * 256:(ot + 1) * 256],
                            t_wt[:, wcol * 128:(wcol + 1) * 128],
                            t_S[:, k, ct, :],
                            start=(ki == 0), stop=(ki == 17))
                ro = opool.tile([128, 2, 256], F32, tag='ro')
                for ot in range(2):
                    nc.scalar.activation(
                        ro[:, ot, :], po[:, ot * 256:(ot + 1) * 256], ACTF.Relu)
                    nc.sync.dma_start(
                        d_out.ap()[ot * 128:(ot + 1) * 128,
                                   cb * 256:(cb + 1) * 256],
                        ro[:, ot, :])

    nc.compile()
    return nc


def _prep_inputs(x, offset, weight):
    """Host-side shard/relayout: per-core input dicts."""
    x = np.asarray(x, np.float32)
    offset = np.asarray(offset, np.float32)
    weight = np.asarray(weight, np.float32)

    # row-pair interleaved, x-padded pixel-major bf16 images
    xqs = []
    yy = np.arange(H)[:, None]
    xx = np.arange(W)[None, :]
    s_even = ((66 * (yy + 1) + xx + 1) * 2).ravel()
    for n in range(N):
        pix = x[n].transpose(1, 2, 0).reshape(H * W, C).astype(ml_dtypes.bfloat16)
        xq = np.zeros((XREC, C), ml_dtypes.bfloat16)
        xq[s_even] = pix                    # record (y, x, r=0) = pixel (y, x)
        xq[s_even - 131] = pix              # record (y-1, x, r=1) = pixel (y, x)
        xqs.append(xq.reshape(-1))

    # weights: wt[c_lo, (k, ct, ot, o_lo)]
    wr = weight.reshape(2, 128, 2, 128, K2)       # [ot, o_lo, ct, c_lo, k]
    wt_host = np.ascontiguousarray(
        wr.transpose(3, 4, 2, 0, 1).reshape(128, K2 * 2 * 2 * 128)
    ).astype(ml_dtypes.bfloat16)

    p = np.arange(128)
    maskBK = np.zeros((128, 18, 128), ml_dtypes.bfloat16)
    maskBK[p, :, p] = 1.0

    # layout-A grids: p = px%128, col = jg*9 + k (jg = px//128, 16 per core)
    cA = np.arange(144)
    jgA, kA = cA // K2, cA % K2
    kyA, kxA = kA // 3 - 1, kA % 3 - 1
    pxA = jgA[None, :] * 128 + p[:, None]
    wA = pxA % W
    rowA = pxA // W
    bAx = (wA + kxA[None, :] + 16.0).astype(np.float32)

    # layout-B grids: P = (kl*2+jh)*16 + q; C = cb*24 + kt*8 + pl
    uB, qB = p // 16, p % 16
    klB, jhB = np.minimum(uB // 2, 2), uB % 2     # clamp dead rows (u>=6)
    CB = np.arange(192)
    cbB, ktB, plB = CB // 24, (CB % 24) // 8, CB % 8
    kB = ktB * 3 + klB[:, None]
    pxB = cbB[None, :] * 256 + jhB[:, None] * 128 + plB[None, :] * 16 + qB[:, None]
    kyB, kxB = kB // 3 - 1, kB % 3 - 1
    wB = pxB % W
    rowB = pxB // W
    bBx = (wB + kxB + 16.0).astype(np.float32)

    in_maps = []
    for core in range(NCORES):
        img, half = core // 2, core % 2
        h0 = half * ROWSC
        offs = offset[img * H * W + h0 * W: img * H * W + h0 * W + PXC]
        bAy = ((h0 + rowA) + kyA[None, :] + 16.0).astype(np.float32)
        bBy = ((h0 + rowB) + kyB + 16.0).astype(np.float32)
        inA = np.stack([offs[pxA, 2 * kA[None, :]],
                        offs[pxA, 2 * kA[None, :] + 1], bAy, bAx], axis=1)
        inB = np.stack([offs[pxB, 2 * kB],
                        offs[pxB, 2 * kB + 1], bBy, bBx], axis=1)
        in_maps.append({
            'xq': xqs[img],
            'wt': wt_host,
            'inA': np.ascontiguousarray(inA.reshape(128, -1).astype(np.float32)),
            'inB': np.ascontiguousarray(inB.reshape(128, -1).astype(np.float32)),
            'maskBK': maskBK.reshape(128, -1),
        })
    return in_maps


def kernel(x, offset, weight, _run_kwargs=None):
    if 'nc' not in _CACHE:
        _CACHE['nc'] = _build_program()
    nc = _CACHE['nc']
    in_maps = _prep_inputs(x, offset, weight)
    res = bass_utils.run_bass_kernel_spmd(
        nc, in_maps, core_ids=list(range(NCORES)), **(_run_kwargs or {}))
    out = np.empty((N, CO, H, W), np.float32)
    for core in range(NCORES):
        img, half = core // 2, core % 2
        out[img, :, half * ROWSC:(half + 1) * ROWSC, :] = \
            res.results[core]['out'].reshape(CO, ROWSC, W)
    _CACHE['last_result'] = res
    return out
